# revision 11
# baseline (speedup 1.0000x reference)
"""Trainium2 Bass kernel for nn_DecoderCell (GRU-style decoder cell).

Reference computation (per batch row):
    r      = sigmoid(x @ Wr.T + hprev @ Ur.T + c @ Cr.T + br)
    z      = sigmoid(x @ Wz.T + hprev @ Uz.T + c @ Cz.T + bz)
    h_prop = tanh  (x @ Wh.T + (r * hprev) @ Uh.T + c @ Ch.T + bh)
    out    = z * h_prop + (1 - z) * hprev

Shapes: B=8192, IN=1024, H=1024, c is [B, 2H].

Strategy:
  - Data-parallel across 8 NeuronCores: batch shard of 1024 rows per core,
    weights replicated. No collectives.
  - All compute in the "transposed domain": per core we hold
    AT = [x | hprev | c].T  -> [4096, 1024]  (k-major: the contraction dim
    lives on SBUF partitions); gate pre-activations come out as [H, B_s]
    tiles, so the r*hprev product needed by the Uh matmul is produced
    directly in k-major layout and no on-device transposes are required.
  - Mixed precision per matmul term: each of the 9 terms (3 gates x
    {W:x, U:hprev, C:c}) runs either as fp16 matmuls (k-tiles of 128) or
    as fp8e4 (e4m3) matmuls in DoubleRow perf mode, which contracts
    k-pairs of 256 per pass at the same per-instruction cost -> 2x PE
    rate for those terms (measured on HW: 220ns/instr both modes).
    The FP8 term set is chosen so the end-to-end RMS relative error stays
    ~1.4-1.7e-2, under the 2e-2 gate: fp8 error flows through saturating
    sigmoid/tanh nonlinearities, and the r-gate is doubly damped (its
    error passes through sigmoid' and then another matmul + tanh).
  - All weights are prescaled by 16 on host (fp8's min normal is 2^-6;
    Xavier weights have std 0.031, so unscaled e4m3 would put ~40% of
    the mass in subnormals). The activation instruction computes
    func(in*scale + bias), so the descale by 1/16 is fused and free.
  - PE loop per gate: 4 column-quarter groups of 4 PSUM banks each
    (2 j-tiles x 2 batch-slices of 512), accumulating over the gate's
    k-chunks.  The 8-bank PSUM pool double-buffers groups; weight slabs
    stream from HBM through a prefetch pool.
  - A post-schedule BIR pass removes back-to-back identical fp16
    LDWEIGHTS (the PE keeps stationary weights across matmuls; Tile
    emits one per matmul).  DoubleRow LDWEIGHTS are left alone -- the
    hardware faulted when they were dedup'd.
"""

import sys

sys.path.insert(0, "/opt/trn_rl_repo")

import numpy as np
import ml_dtypes
from contextlib import ExitStack

B = 8192
IN = 1024
H = 1024
NCORES = 8
BS = B // NCORES          # batch rows per core
NB = BS // 512            # 512-wide moving slices per core
KSLAB = 4                 # k-tiles (fp16) or k-pairs (fp8) per weight-slab DMA
WSCALE = 16.0             # host-side weight prescale, descaled in activation

# Which terms run as fp8e4 DoubleRow matmuls. Gate -> subset of "WUC"
# (W: x contribution, U: hprev/r*hprev contribution, C: c contribution).
FP8 = {"r": "WUC", "z": "C", "h": "U"}

# term -> (k-tile offset in AT, depth in 128-rows)
TERM_K = {"W": (0, 8), "U": (8, 8), "C": (16, 16)}

# Per-gate term issue order. z leads the kernel and starts on its C term:
# the first matmul then only waits on ~512KB of DMA (one c8 pair + one w8
# slab) while the x16/h16 tiles stream in behind it.
TERM_ORDER = {"r": "WUC", "z": "CWU", "h": "WUC"}

# Group schedule: interleave z and r column-quarters to spread the input
# DMA burst (z's fp16 terms need x16/h16 early; r is fp8 and DMA-light),
# then h (which needs every rh tile, i.e. all of r's epilogues).
GROUPS = [
    ("z", 0), ("r", 0), ("z", 1), ("r", 1),
    ("z", 2), ("r", 2), ("z", 3), ("r", 3),
    ("h", 0), ("h", 1), ("h", 2), ("h", 3),
]

_CACHE = {}


def _dedup_ldweights(nc, mybir):
    """Drop redundant fp16 InstLdweights from the scheduled BIR.

    Tile splits every non-fp32 matmul into an explicit LDWEIGHTS + MATMUL
    pair, and the walrus invocation runs with --enable-ldw-opt=false, so
    back-to-back matmuls that reuse the same stationary tile each pay a
    full weight reload. The PE array keeps its weights across matmuls, so
    an LDWEIGHTS identical to the immediately preceding one (and carrying
    no semaphore waits or updates) is a no-op -- remove it.  DoubleRow
    (fp8 perf-mode) LDWEIGHTS are never removed: doing so wedged the PE
    (NRT_EXEC_UNIT_UNRECOVERABLE).
    """
    removed = 0
    for f in nc.m.functions:
        for bb in f.blocks:
            keep = []
            prev_sig = None
            for inst in bb.instructions:
                tn = type(inst).__name__
                if getattr(inst, "engine", None) == mybir.EngineType.PE:
                    if tn == "InstLdweights":
                        if getattr(inst, "perf_mode", None) is not None:
                            prev_sig = None
                            keep.append(inst)
                            continue
                        sig = str(inst.ins[0]) if inst.ins else None
                        si = inst.sync_info
                        clean = si is None or (
                            len(si.on_wait) == 0 and len(si.on_update) == 0
                        )
                        if sig is not None and sig == prev_sig and clean:
                            removed += 1
                            continue
                        prev_sig = sig
                    elif tn in ("InstMatmult", "InstEventSemaphore", "InstNoOp"):
                        pass  # these don't disturb the loaded weights
                    else:
                        prev_sig = None
                keep.append(inst)
            bb.instructions[:] = keep
    return removed


def _build_nc():
    import concourse.bacc as bacc
    import concourse.tile as tile
    from concourse import mybir

    f32 = mybir.dt.float32
    f16 = mybir.dt.float16
    f8 = mybir.dt.float8e4
    DR = mybir.MatmulPerfMode.DoubleRow
    SIG = mybir.ActivationFunctionType.Sigmoid
    TANH = mybir.ActivationFunctionType.Tanh

    nc = bacc.Bacc("TRN2", target_bir_lowering=False, debug=False)

    at16_d = nc.dram_tensor("at16", [4 * IN, BS], f16, kind="ExternalInput")
    at8_d = nc.dram_tensor("at8", [4 * IN, BS], f8, kind="ExternalInput")
    w_d = {}
    for g in "rzh":
        for t in "WUC":
            depth = TERM_K[t][1] * 128
            dt = f8 if t in FP8[g] else f16
            w_d[(g, t)] = nc.dram_tensor(
                f"w{g}{t}", [depth, H], dt, kind="ExternalInput"
            )
    b_d = {
        g: nc.dram_tensor(f"b{g}", [128, 8], f32, kind="ExternalInput")
        for g in "rzh"
    }
    # fp16 output: halves the tail DMA drain; adds only ~2e-4 RMS rounding
    out_d = nc.dram_tensor("out_t", [H, BS], f16, kind="ExternalOutput")

    hU8 = "U" in FP8["h"]

    with tile.TileContext(nc) as tc:
        with ExitStack() as ctx:
            pp = ctx.enter_context(tc.tile_pool(name="persist", bufs=1))
            wp = ctx.enter_context(tc.tile_pool(name="wslab", bufs=8))
            rp = ctx.enter_context(tc.tile_pool(name="rtmp", bufs=4))
            hpp = ctx.enter_context(tc.tile_pool(name="hprop", bufs=4))
            op = ctx.enter_context(tc.tile_pool(name="otile", bufs=4))
            psp = ctx.enter_context(tc.tile_pool(name="ps", bufs=8, space="PSUM"))

            at16_t = [
                pp.tile([128, BS], f16, tag=f"at{k}", name=f"at{k}")
                for k in range(32)
            ]
            a8_t = [
                pp.tile([128, 2, BS], f8, tag=f"a8_{p}", name=f"a8_{p}")
                for p in range(16)
            ]
            if hU8:
                rh8_t = [
                    pp.tile([128, 2, BS], f8, tag=f"rh{q}", name=f"rh{q}")
                    for q in range(4)
                ]
            else:
                rh16_t = [
                    pp.tile([128, BS], f16, tag=f"rh{j}", name=f"rh{j}")
                    for j in range(8)
                ]
            z_t = [
                [pp.tile([128, 512], f16, tag=f"z{j}_{b}", name=f"z{j}_{b}") for b in range(NB)]
                for j in range(8)
            ]
            # w = (1-z)*hprev, precomputed at the z epilogue so the h
            # epilogue (the kernel's tail) is only mul+add
            w_t = [
                [pp.tile([128, 512], f16, tag=f"wz{j}_{b}", name=f"wz{j}_{b}") for b in range(NB)]
                for j in range(8)
            ]
            bias_t = {g: pp.tile([128, 8], f32, tag=f"bias{g}", name=f"bias{g}") for g in "rzh"}

            at16_dma = [None] * 32
            a8_dma = [None] * 16

            # input tensors stream on the Activation engine's DMA queue so
            # they don't sit behind the weight-slab stream on SP's queue
            def ensure_a16(k):
                if at16_dma[k] is None:
                    at16_dma[k] = nc.scalar.dma_start(
                        at16_t[k][:], at16_d.ap()[k * 128:(k + 1) * 128, :]
                    )
                return at16_dma[k]

            def ensure_a8(p):
                if a8_dma[p] is None:
                    src = at8_d.ap()[p * 256:(p + 1) * 256, :].rearrange(
                        "(two q) b -> q two b", q=128
                    )
                    a8_dma[p] = nc.scalar.dma_start(a8_t[p][:], src)
                return a8_dma[p]

            bias_loaded = [False]

            def ensure_bias():
                # biases are first needed at the first epilogue; keep them
                # out of the critical DMA prefix
                if not bias_loaded[0]:
                    for g in "rzh":
                        nc.scalar.dma_start(bias_t[g][:], b_d[g].ap()[:, :])
                    bias_loaded[0] = True

            def do_group(g, jq):
                # output tiles: j in [jq*128*2, ...), all BS batch cols
                ps = {}
                for jl in range(2):
                    for b in range(NB):
                        ps[(jl, b)] = psp.tile([128, 512], f32, tag="ps",
                                               name=f"ps_{g}_{jq}_{jl}_{b}")
                # count matmul units (one matmul per (jl, b) each)
                nu = sum(
                    TERM_K[t][1] // (2 if t in FP8[g] else 1) for t in "WUC"
                )
                ui = 0
                for t in TERM_ORDER[g]:
                    koff, dep = TERM_K[t]
                    is8 = t in FP8[g]
                    if is8:
                        npair = dep // 2
                        poff = koff // 2
                        for ss in range((npair + KSLAB - 1) // KSLAB):
                            p0 = ss * KSLAB
                            sl = min(KSLAB, npair - p0)
                            slab = wp.tile([128, sl, 2, 256], f8, tag="w8",
                                           name=f"w8_{g}{t}_{jq}_{ss}")
                            src = w_d[(g, t)].ap()[
                                p0 * 256:(p0 + sl) * 256,
                                jq * 256:(jq + 1) * 256,
                            ].rearrange("(a two p) j -> p a two j", p=128, two=2)
                            nc.sync.dma_start(slab[:], src)
                            for dp in range(sl):
                                pl = p0 + dp
                                if g == "h" and t == "U":
                                    mov = rh8_t[pl]
                                else:
                                    ensure_a8(poff + pl)
                                    mov = a8_t[poff + pl]
                                for jl in range(2):
                                    lhsT = slab[:, dp, :, jl * 128:(jl + 1) * 128]
                                    for b in range(NB):
                                        nc.tensor.matmul(
                                            ps[(jl, b)][:],
                                            lhsT,
                                            mov[:, :, b * 512:(b + 1) * 512],
                                            start=(ui == 0),
                                            stop=(ui == nu - 1),
                                            perf_mode=DR,
                                        )
                                ui += 1
                    else:
                        for ks in range((dep + KSLAB - 1) // KSLAB):
                            k0 = ks * KSLAB
                            sl = min(KSLAB, dep - k0)
                            slab = wp.tile([128, sl, 256], f16, tag="w16",
                                           name=f"w16_{g}{t}_{jq}_{ks}")
                            src = w_d[(g, t)].ap()[
                                k0 * 128:(k0 + sl) * 128,
                                jq * 256:(jq + 1) * 256,
                            ].rearrange("(a p) j -> p a j", p=128)
                            nc.sync.dma_start(slab[:], src)
                            for dk in range(sl):
                                kl = k0 + dk
                                if g == "h" and t == "U":
                                    mov = rh16_t[kl]
                                else:
                                    ensure_a16(koff + kl)
                                    mov = at16_t[koff + kl]
                                for jl in range(2):
                                    lhsT = slab[:, dk, jl * 128:(jl + 1) * 128]
                                    for b in range(NB):
                                        nc.tensor.matmul(
                                            ps[(jl, b)][:],
                                            lhsT,
                                            mov[:, b * 512:(b + 1) * 512],
                                            start=(ui == 0),
                                            stop=(ui == nu - 1),
                                        )
                                ui += 1
                assert ui == nu
                ensure_bias()
                for jl in range(2):
                    jt = 2 * jq + jl
                    for b in range(NB):
                        pst = ps[(jl, b)]
                        bias_ap = bias_t[g][:, jt:jt + 1]
                        bsl = slice(b * 512, (b + 1) * 512)
                        if g == "r":
                            ensure_a16(8 + jt)
                            tmp = rp.tile([128, 512], f32, tag="rt", name=f"rt_{jt}_{b}")
                            nc.scalar.activation(tmp[:], pst[:], SIG,
                                                 bias=bias_ap, scale=1.0 / WSCALE)
                            if hU8:
                                dst = rh8_t[jt // 2][:, jt % 2, bsl]
                            else:
                                dst = rh16_t[jt][:, bsl]
                            nc.vector.tensor_mul(dst, tmp[:], at16_t[8 + jt][:, bsl])
                        elif g == "z":
                            ensure_a16(8 + jt)
                            nc.scalar.activation(z_t[jt][b][:], pst[:], SIG,
                                                 bias=bias_ap, scale=1.0 / WSCALE)
                            hT = at16_t[8 + jt][:, bsl]
                            tmp = rp.tile([128, 512], f32, tag="rt", name=f"zh_{jt}_{b}")
                            # w = (1-z)*h = h - z*h
                            nc.vector.tensor_mul(tmp[:], z_t[jt][b][:], hT)
                            nc.vector.tensor_sub(w_t[jt][b][:], hT, tmp[:])
                        else:
                            hp = hpp.tile([128, 512], f32, tag="hp", name=f"hp_{jt}_{b}")
                            nc.scalar.activation(hp[:], pst[:], TANH,
                                                 bias=bias_ap, scale=1.0 / WSCALE)
                            ot = op.tile([128, 512], f16, tag="ot", name=f"ot_{jt}_{b}")
                            zp = rp.tile([128, 512], f32, tag="rt", name=f"zp_{jt}_{b}")
                            # out = z*hp + (1-z)*h, with (1-z)*h precomputed
                            nc.vector.tensor_mul(zp[:], z_t[jt][b][:], hp[:])
                            nc.vector.tensor_add(ot[:], zp[:], w_t[jt][b][:])
                            nc.sync.dma_start(
                                out_d.ap()[jt * 128:(jt + 1) * 128, bsl], ot[:]
                            )

            for g, jq in GROUPS:
                do_group(g, jq)

    _dedup_ldweights(nc, mybir)
    nc.finalize()
    return nc


def _get_nc():
    if "nc" not in _CACHE:
        _CACHE["nc"] = _build_nc()
    return _CACHE["nc"]


def _host_prep(inputs):
    x = np.asarray(inputs["x"], dtype=np.float32)
    hprev = np.asarray(inputs["hprev"], dtype=np.float32)
    c = np.asarray(inputs["c"], dtype=np.float32)
    A = np.concatenate([x, hprev, c], axis=1)                  # [B, 4096]
    AF = np.ascontiguousarray(A.T)                             # [4096, B]
    at16 = AF.astype(np.float16)
    at8 = AF.astype(ml_dtypes.float8_e4m3)
    wnames = {
        ("r", "W"): "Wr", ("r", "U"): "Ur", ("r", "C"): "Cr",
        ("z", "W"): "Wz", ("z", "U"): "Uz", ("z", "C"): "Cz",
        ("h", "W"): "Wh", ("h", "U"): "Uh", ("h", "C"): "Ch",
    }
    w = {}
    for (g, t), nm in wnames.items():
        M = np.ascontiguousarray(np.asarray(inputs[nm], np.float32).T) * WSCALE
        if t in FP8[g]:
            w[(g, t)] = M.astype(ml_dtypes.float8_e4m3)
        else:
            w[(g, t)] = M.astype(np.float16)
    bias = {
        g: np.ascontiguousarray(
            np.asarray(inputs["b" + g], dtype=np.float32).reshape(8, 128).T
        )
        for g in "rzh"
    }
    return at16, at8, w, bias


def _in_maps(inputs):
    at16, at8, w, bias = _host_prep(inputs)
    maps = []
    for s in range(NCORES):
        m = {
            "at16": np.ascontiguousarray(at16[:, s * BS:(s + 1) * BS]),
            "at8": np.ascontiguousarray(at8[:, s * BS:(s + 1) * BS]),
            "br": bias["r"],
            "bz": bias["z"],
            "bh": bias["h"],
        }
        for (g, t), M in w.items():
            m[f"w{g}{t}"] = M
        maps.append(m)
    return maps


def run_device(inputs, trace=False, **kwargs):
    """Run the SPMD kernel; returns (full_output, BassKernelResults)."""
    from concourse.bass_utils import run_bass_kernel_spmd

    nc = _get_nc()
    res = run_bass_kernel_spmd(
        nc, _in_maps(inputs), core_ids=list(range(NCORES)), trace=trace, **kwargs
    )
    out = np.empty((B, H), dtype=np.float32)
    for s in range(NCORES):
        out[s * BS:(s + 1) * BS, :] = res.results[s]["out_t"].T.astype(np.float32)
    return out, res


def kernel(**inputs):
    out, _ = run_device(inputs, trace=False)
    return out
